# revision 1
# baseline (speedup 1.0000x reference)
# Trainium2 Bass kernel for nn_MipMap (embedding_lookup).
#
# Math: fused(p) = sum_lvl b_lvl * bilinear_interp(mip_lvl, p) is linear in
# the grids, so all 6 mip levels are pre-combined into ONE grid
# G = sum_lvl b_lvl * (C_lvl @ X @ C_lvl^T)  (C_lvl = 256x256 banded
# reflect-padded Gaussian conv matrix, baked at trace time).  Each point then
# needs a single 4-corner bilinear gather from G followed by the 2-layer MLP.
#
# Data-parallel over points: 8 cores, each runs the identical program on a
# 32768-point shard; the grid preprocessing is replicated on every core.
#
# Layouts:
#   pass1 (axis-0 conv): PE matmuls, out[i,(j,f)], intermediate -> DRAM fp16
#   pass2 (axis-1 conv): PE matmuls with even/odd j'-pair packing (clean
#          512B DRAM reads), all levels + b0*X accumulated in PSUM,
#          output = G^T [j, i, f] fp16 in DRAM
#   points: indirect-DMA gather of 2-cell strips (512B) per (point, corner-j),
#          scalar_tensor_tensor bilinear combine -> feat[pt,f] fp16,
#          xbar DMA transpose -> [f,pt], PE W1 matmul, ACT bias+relu,
#          PE W2 matmul -> psum[pt,4], one clean output DMA.

import numpy as np

R = 256
F = 128
N_PTS = 262144
N_CORES = 8
PTS_CORE = N_PTS // N_CORES          # 32768
PT_COLS = PTS_CORE // 128            # 256 point-tiles of 128
LEVELS = [1, 2, 3, 4, 5]             # blurred levels (level 0 = X itself)

# pass1 output M-tiles (rows of output) and their input K-windows
P1_TILES = [(0, 96, 0, 112), (96, 96, 80, 128), (192, 64, 176, 80)]
# (out_start, out_len, win_start, win_len)

GATHER_GROUP = 16                    # point-tiles per indirect gather


def _gaussian_kernel(M, std):
    n = np.arange(M, dtype=np.float64) - (M - 1) / 2.0
    w = np.exp(-0.5 * (n / std) ** 2)
    return (w / w.sum()).astype(np.float32)


def _conv_matrix(s):
    """A[z, m]: out[z] = sum_m A[z, m] x[m] for the reference's
    reflect-pad(s) + conv_same(kern s) + crop(s) pipeline."""
    kern = _gaussian_kernel(s, s / 2.0).astype(np.float64)
    A = np.zeros((R, R), np.float64)
    for z in range(R):
        for k in range(s):
            m = z - s // 2 + k
            if m < 0:
                m = -m
            elif m > R - 1:
                m = 2 * (R - 1) - m
            A[z, m] += kern[k]
    return A


def _build_consts(b_levels):
    """Numpy constants shipped as kernel inputs (fp16)."""
    mats = {l: _conv_matrix(2 ** l) for l in LEVELS}
    # pass1 lhsT blocks: [5, 3, 128, 96]  (lvl, mtile, K(win rows), M(out rows))
    c1 = np.zeros((5, 3, 128, 96), np.float64)
    for li, l in enumerate(LEVELS):
        A = mats[l]
        for w, (o0, olen, w0, wlen) in enumerate(P1_TILES):
            c1[li, w, :wlen, :olen] = A[o0:o0 + olen, w0:w0 + wlen].T
    # pass2 lhsT blocks: [5, 2, 2, 128, 128] (lvl, eo, jtile, K(pairs), M(j))
    d2 = np.zeros((5, 2, 2, 128, 128), np.float64)
    for li, l in enumerate(LEVELS):
        A = mats[l] * float(b_levels[li + 1])
        for e in range(2):
            for jt in range(2):
                # element [p, j] = A[jt*128 + j, 2p + e]
                d2[li, e, jt] = A[jt * 128:(jt + 1) * 128, e::2].T
    return c1.astype(np.float16), d2.astype(np.float16)


def _numpy_model(pt, X, b_levels, W1, b1, W2, b2):
    """Pure-numpy mirror of the device algorithm (for validation)."""
    X = np.asarray(X, np.float32)
    interm = [np.asarray(X, np.float16).astype(np.float32)]
    for l in LEVELS:
        A = _conv_matrix(2 ** l).astype(np.float32)
        Y = np.einsum("zm,mjf->zjf", A, X).astype(np.float16).astype(np.float32)
        interm.append(Y)
    G = b_levels[0] * interm[0]
    for li, l in enumerate(LEVELS):
        A = _conv_matrix(2 ** l).astype(np.float32)
        G = G + b_levels[li + 1] * np.einsum("wn,inf->iwf", A, interm[li + 1])
    GT = np.ascontiguousarray(G.transpose(1, 0, 2)).astype(np.float16)  # [j,i,f]
    af = (np.asarray(pt, np.float32) + 1.0) * 127.5
    fr = np.mod(af, 1.0)
    fl = (af - fr).astype(np.int32)
    c0, c1i = fl[:, 0], fl[:, 1]
    f0, f1 = fr[:, 0], fr[:, 1]
    cA0 = GT[c1i, c0]
    cA1 = GT[c1i, c0 + 1]
    cB0 = GT[c1i + 1, c0]
    cB1 = GT[c1i + 1, c0 + 1]
    w00 = ((1 - f1) * (1 - f0))[:, None].astype(np.float32)
    w01 = ((1 - f1) * f0)[:, None].astype(np.float32)
    w10 = (f1 * (1 - f0))[:, None].astype(np.float32)
    w11 = (f1 * f0)[:, None].astype(np.float32)
    feat = (cA0 * w00 + cA1 * w01 + cB0 * w10 + cB1 * w11).astype(np.float16)
    z = feat.astype(np.float32) @ np.asarray(W1, np.float32) + b1
    h = np.maximum(z, 0.0)
    return h @ np.asarray(W2, np.float32) + b2


def _build_bass():
    import concourse.bass as bass
    import concourse.mybir as mybir
    import concourse.tile as tile
    from concourse import bacc
    from concourse.masks import make_identity

    f32 = mybir.dt.float32
    f16 = mybir.dt.float16
    i32 = mybir.dt.int32
    Alu = mybir.AluOpType
    Act = mybir.ActivationFunctionType
    dep = tile.add_dep_helper

    def _absorb(nc_, *insts):
        pn = nc_.gpsimd.engine_nop()
        for i in insts:
            if i is not None:
                dep(pn.ins, i.ins, reason="absorb DMA deps on Pool")
        return pn

    def _after(pn, dma_inst):
        dep(dma_inst.ins, pn.ins, sync=False, reason="order after absorber")
        return dma_inst

    nc = bacc.Bacc()
    x = nc.dram_tensor("x", [R, R, F], f32, kind="ExternalInput")
    ptx = nc.dram_tensor("ptx", [PTS_CORE, 2], f32, kind="ExternalInput")
    w1d = nc.dram_tensor("w1d", [F, 128], f32, kind="ExternalInput")
    b1d = nc.dram_tensor("b1d", [128], f32, kind="ExternalInput")
    w2d = nc.dram_tensor("w2d", [128, 4], f32, kind="ExternalInput")
    b2d = nc.dram_tensor("b2d", [4], f32, kind="ExternalInput")
    c1d = nc.dram_tensor("c1d", [5, 3, 128, 96], f16, kind="ExternalInput")
    d2d = nc.dram_tensor("d2d", [5, 2, 2, 128, 128], f16, kind="ExternalInput")
    outd = nc.dram_tensor("out", [PTS_CORE, 4], f32, kind="ExternalOutput")
    gdump = (nc.dram_tensor("gdump", [R, R, F], f16, kind="ExternalOutput")
             if DEBUG_DUMP_GRID[0] else None)
    if DEBUG_DUMP_GRID[0]:
        cdump = nc.dram_tensor("cdump", [128, PT_COLS], mybir.dt.int32,
                               kind="ExternalOutput")
        frdump = nc.dram_tensor("frdump", [128, PT_COLS * 2], f32,
                                kind="ExternalOutput")
        sdump = nc.dram_tensor("sdump", [128, GATHER_GROUP, 2 * F], f16,
                               kind="ExternalOutput")
    else:
        cdump = frdump = sdump = None

    with tile.TileContext(nc) as tc:
        with (
            tc.tile_pool(name="dram", bufs=1, space="DRAM") as dpool,
            tc.tile_pool(name="consts", bufs=1) as cpool,
        ):
            # per-region intermediate DRAM tiles: single writer + single
            # reader-dep each, so every DMA touching them needs <=1 wait
            interm = {}
            for li in range(5):
                for w, (o0, olen, _, _) in enumerate(P1_TILES):
                    for h in range(2):
                        t = dpool.tile([olen, 128, F], f16,
                                       name=f"im_{li}_{w}_{h}",
                                       tag=f"im_{li}_{w}_{h}")
                        interm[(li, w, h)] = t
            gridT = dpool.tile([R, R, F], f16)

            # ---- constants to SBUF ----
            c1_sb = []
            for li in range(5):
                row = []
                for w in range(3):
                    t = cpool.tile([128, 96], f16, name=f"c1_{li}_{w}",
                                   tag=f"c1_{li}_{w}")
                    nc.gpsimd.dma_start(t, c1d[li, w])
                    row.append(t)
                c1_sb.append(row)
            d2_sb = []
            for li in range(5):
                rows = []
                for e in range(2):
                    row = []
                    for jt in range(2):
                        t = cpool.tile([128, 128], f16,
                                       name=f"d2_{li}_{e}_{jt}",
                                       tag=f"d2_{li}_{e}_{jt}")
                        nc.gpsimd.dma_start(t, d2d[li, e, jt])
                        row.append(t)
                    rows.append(row)
                d2_sb.append(rows)
            w1_sb = cpool.tile([128, 128], f16, tag="w1_sb")
            nc.gpsimd.dma_start(w1_sb, w1d[:, :])   # fp32 -> fp16 cast DMA
            w2_sb = cpool.tile([128, 4], f16, tag="w2_sb")
            nc.gpsimd.dma_start(w2_sb, w2d[:, :])
            b1_sb = cpool.tile([128, 1], f32, tag="b1_sb")
            nc.gpsimd.dma_start(b1_sb, b1d.ap().rearrange("(h o) -> h o", o=1))
            b2_sb = cpool.tile([128, 4], f32, tag="b2_sb")
            b2_bcast = bass.AP(tensor=b2d.ap().tensor, offset=0,
                               ap=[[0, 128], [1, 4]])
            nc.gpsimd.dma_start(b2_sb, b2_bcast)
            ident = cpool.tile([128, 128], f16, tag="ident")
            make_identity(nc, ident)

            # ================= pass 1: axis-0 conv =================
            store_inst = {}
            for sess, wlist in (("a", (0, 1)), ("b", (2,))):
                with (
                    tc.tile_pool(name=f"p1x{sess}", bufs=1) as xpool,
                    tc.tile_pool(name=f"p1s{sess}", bufs=2) as spool,
                    tc.tile_pool(name=f"p1p{sess}", bufs=4, space="PSUM")
                        as ppool,
                ):
                    xw = {}
                    for w in wlist:
                        o0, olen, w0, wlen = P1_TILES[w]
                        t = xpool.tile([wlen, R * F], f16, name=f"xw{w}",
                                       tag=f"xw{w}")
                        if sess == "b":
                            # shield: absorb released-pool deps on Pool engine
                            nc.gpsimd.memset(t, 0.0)
                        nc.gpsimd.dma_start(
                            t, x[w0:w0 + wlen].rearrange("i j f -> i (j f)"))
                        xw[w] = t
                    for w in wlist:
                        o0, olen, w0, wlen = P1_TILES[w]
                        for li in range(5):
                            for h in range(2):      # j'-halves: 16384 each
                                use_dve = (li + w + h) % 2 == 0
                                stg = spool.tile([96, 16384], f16, tag="stg")
                                last_ev = None
                                for cch in range(32):
                                    ps = ppool.tile([128, 512], f32, tag="ps")
                                    nc.tensor.matmul(
                                        ps[:olen],
                                        lhsT=c1_sb[li][w][:wlen, :olen],
                                        rhs=xw[w][:, h * 16384 + cch * 512:
                                                  h * 16384 + (cch + 1) * 512],
                                        start=True, stop=True)
                                    dst = stg[:olen, cch * 512:(cch + 1) * 512]
                                    if use_dve:
                                        last_ev = nc.vector.tensor_copy(
                                            dst, ps[:olen])
                                    else:
                                        last_ev = nc.scalar.activation(
                                            dst, ps[:olen], Act.Copy)
                                pn = _absorb(nc, last_ev)
                                st = nc.gpsimd.dma_start(
                                    interm[(li, w, h)]
                                    .rearrange("i j f -> i (j f)"),
                                    stg[:olen])
                                _after(pn, st)
                                store_inst[(li, w, h)] = st

            # ================= pass 2: axis-1 conv, emit G^T =================
            with (
                tc.tile_pool(name="p2rh", bufs=3) as rhpool,
                tc.tile_pool(name="p2xc", bufs=3) as xcpool,
                tc.tile_pool(name="p2g", bufs=1) as gpool,
                tc.tile_pool(name="p2p", bufs=4, space="PSUM") as p2p,
            ):
                gstage = [gpool.tile([128, R * F], f16,
                                     name=f"gstage{jt}", tag=f"gstage{jt}")
                          for jt in range(2)]
                last_mm_of_blk = {}
                last_stt_of_blk = {}
                for blk in range(64):               # i-blocks of 4 rows
                    i0 = blk * 4
                    w = 0 if i0 < 96 else (1 if i0 < 192 else 2)
                    o0 = P1_TILES[w][0]
                    # absorber: interm-store lanes + PE/DVE slot-reuse ticks
                    pdeps = [store_inst[(li, w, h)]
                             for li in range(5) for h in range(2)]
                    if blk >= 3:
                        pdeps.append(last_mm_of_blk[blk - 3])
                        pdeps.append(last_stt_of_blk[blk - 3])
                    pn = _absorb(nc, *pdeps)
                    rhs_lv = []
                    for li in range(5):
                        rh = rhpool.tile([128, 1024], f16, tag=f"rh{li}")
                        rh4 = rh.rearrange("p (i e f) -> p i e f", e=2, f=F)
                        if blk < 3:
                            nc.gpsimd.memset(rh, 0.0)   # zone shield
                        for h in range(2):
                            it = interm[(li, w, h)]
                            src_p = bass.AP(
                                tensor=it.tensor,
                                offset=it.offset + (i0 - o0) * 128 * F,
                                ap=[[2 * F, 64], [128 * F, 4], [F, 2],
                                    [1, F]])
                            ld = nc.gpsimd.dma_start(
                                rh4[:, :, :, :][h * 64:(h + 1) * 64], src_p)
                            _after(pn, ld)
                        rhs_lv.append(rh)
                    for jt in range(2):
                        xc = xcpool.tile([128, 512], f16, tag="xc")
                        if blk < 3:
                            nc.gpsimd.memset(xc, 0.0)   # zone shield
                        xs = x[i0:i0 + 4, jt * 128:(jt + 1) * 128]
                        xs_p = bass.AP(
                            tensor=xs.tensor, offset=xs.offset,
                            ap=[[F, 128], [R * F, 4], [1, F]])
                        _after(pn, nc.gpsimd.dma_start(
                            xc.rearrange("p (i f) -> p i f", f=F), xs_p))
                        ps = p2p.tile([128, 512], f32, tag="gps")
                        n_mm = 10
                        k = 0
                        for li in range(5):
                            rh3 = rhs_lv[li].rearrange(
                                "p (i e f) -> p i e f", e=2, f=F)
                            for e in range(2):
                                mm = nc.tensor.matmul(
                                    ps,
                                    lhsT=d2_sb[li][e][jt],
                                    rhs=rh3[:, :, e],
                                    start=(k == 0), stop=(k == n_mm - 1))
                                k += 1
                        last_mm_of_blk[blk] = mm
                        last_stt_of_blk[blk] = nc.vector.scalar_tensor_tensor(
                            out=gstage[jt][:, blk * 512:(blk + 1) * 512],
                            in0=xc, scalar=B_LEVEL0[0],
                            in1=ps, op0=Alu.mult, op1=Alu.add)
                pns = _absorb(nc, last_stt_of_blk[63])
                st1 = _after(pns, nc.gpsimd.dma_start(
                    gridT[128:256].rearrange("j i f -> j (i f)"), gstage[1]))
                pns0 = _absorb(nc, st1)
                st0 = _after(pns0, nc.gpsimd.dma_start(
                    gridT[0:128].rearrange("j i f -> j (i f)"), gstage[0]))

            # ================= point phase =================
            with (
                tc.tile_pool(name="ptw", bufs=1) as wpool,
                tc.tile_pool(name="strips", bufs=2) as stpool,
                tc.tile_pool(name="feat", bufs=3) as fpool,
                tc.tile_pool(name="ptp", bufs=2, space="PSUM") as ptp,
                tc.tile_pool(name="ptt", bufs=2, space="PSUM") as ptt,
                tc.tile_pool(name="ptp4", bufs=2, space="PSUM") as ptp4,
            ):
                pt_sb = wpool.tile([128, PT_COLS * 2], f32, tag="pt_sb")
                nc.gpsimd.memset(pt_sb, 0.0)        # zone shield
                nc.gpsimd.dma_start(
                    pt_sb, ptx.ap().rearrange("(p t) c -> p (t c)", p=128))
                af = wpool.tile([128, PT_COLS * 2], f32, tag="af")
                nc.vector.tensor_scalar(af, pt_sb, 1.0, 127.5,
                                        Alu.add, Alu.mult)
                # floor/frac without mod: round-to-nearest int cast, then fix
                # up the d<0 cases exactly
                il0 = wpool.tile([128, PT_COLS * 2], i32, tag="il0")
                nc.vector.tensor_copy(il0, af)
                ilf = wpool.tile([128, PT_COLS * 2], f32, tag="ilf")
                nc.vector.tensor_copy(ilf, il0)
                dd = wpool.tile([128, PT_COLS * 2], f32, tag="dd")
                nc.vector.tensor_tensor(dd, af, ilf, Alu.subtract)
                neg = wpool.tile([128, PT_COLS * 2], f32, tag="neg")
                nc.vector.tensor_scalar(neg, dd, 0.0, None, Alu.is_lt)
                fr = wpool.tile([128, PT_COLS * 2], f32, tag="fr")
                nc.vector.tensor_tensor(fr, dd, neg, Alu.add)
                flf = wpool.tile([128, PT_COLS * 2], f32, tag="flf")
                nc.vector.tensor_tensor(flf, ilf, neg, Alu.subtract)
                il = wpool.tile([128, PT_COLS * 2], i32, tag="il")
                nc.vector.tensor_copy(il, flf)
                il3 = il.rearrange("p (t c) -> p t c", c=2)
                fr3 = fr.rearrange("p (t c) -> p t c", c=2)
                cellA = wpool.tile([128, PT_COLS], i32, tag="cellA")
                nc.vector.tensor_scalar(cellA, il3[:, :, 1], 256, None,
                                        Alu.mult)
                nc.vector.tensor_tensor(cellA, cellA, il3[:, :, 0], Alu.add)
                cellB = wpool.tile([128, PT_COLS], i32, tag="cellB")
                nc.vector.tensor_scalar(cellB, cellA, 256, None, Alu.add)
                g0 = wpool.tile([128, PT_COLS], f32, tag="g0")
                nc.vector.tensor_scalar(g0, fr3[:, :, 0], -1.0, 1.0,
                                        Alu.mult, Alu.add)
                g1 = wpool.tile([128, PT_COLS], f32, tag="g1")
                nc.vector.tensor_scalar(g1, fr3[:, :, 1], -1.0, 1.0,
                                        Alu.mult, Alu.add)
                w00 = wpool.tile([128, PT_COLS], f32, tag="w00")
                nc.vector.tensor_tensor(w00, g1, g0, Alu.mult)
                w01 = wpool.tile([128, PT_COLS], f32, tag="w01")
                nc.vector.tensor_tensor(w01, g1, fr3[:, :, 0], Alu.mult)
                w10 = wpool.tile([128, PT_COLS], f32, tag="w10")
                nc.vector.tensor_tensor(w10, fr3[:, :, 1], g0, Alu.mult)
                w11 = wpool.tile([128, PT_COLS], f32, tag="w11")
                w11_i = nc.vector.tensor_tensor(w11, fr3[:, :, 1],
                                                fr3[:, :, 0], Alu.mult)
                osb = wpool.tile([128, PT_COLS * 4], f32, tag="osb")

                gridF = gridT.rearrange("j i f -> (j i) f")
                # fence on Pool engine: wait for both grid stores once, so the
                # gathers' RAW on gridT is elided afterwards
                pf = _absorb(nc, st0, st1, w11_i)
                n_groups = PT_COLS // GATHER_GROUP
                ph = None
                last_comb_of_group = {}
                for g in range(n_groups):
                    t0 = g * GATHER_GROUP
                    stA = stpool.tile([128, GATHER_GROUP, 2 * F], f16,
                                      tag="stA")
                    stB = stpool.tile([128, GATHER_GROUP, 2 * F], f16,
                                      tag="stB")
                    if g < 2:
                        nc.gpsimd.memset(stA, 0.0)  # zone shield
                        nc.gpsimd.memset(stB, 0.0)
                        png = pf
                    else:
                        png = _absorb(nc, last_comb_of_group[g - 2])
                    if g == 0:
                        stpool_first_stA = stA
                    for s_i in range(GATHER_GROUP):
                        gA = _after(png, nc.gpsimd.indirect_dma_start(
                            out=stA[:, s_i, :], out_offset=None,
                            in_=gridF[:, :],
                            in_offset=bass.IndirectOffsetOnAxis(
                                ap=cellA[:, t0 + s_i:t0 + s_i + 1], axis=0)))
                        gB = _after(png, nc.gpsimd.indirect_dma_start(
                            out=stB[:, s_i, :], out_offset=None,
                            in_=gridF[:, :],
                            in_offset=bass.IndirectOffsetOnAxis(
                                ap=cellB[:, t0 + s_i:t0 + s_i + 1], axis=0)))
                    for s in range(GATHER_GROUP):
                        t = t0 + s
                        fa = fpool.tile([128, 128], f16, tag="fa")
                        nc.vector.tensor_scalar(fa, stA[:, s, 0:F],
                                                w00[:, t:t + 1], None,
                                                Alu.mult)
                        fb = fpool.tile([128, 128], f16, tag="fb")
                        nc.vector.scalar_tensor_tensor(
                            out=fb, in0=stA[:, s, F:2 * F],
                            scalar=w01[:, t:t + 1], in1=fa,
                            op0=Alu.mult, op1=Alu.add)
                        fc = fpool.tile([128, 128], f16, tag="fc")
                        nc.vector.scalar_tensor_tensor(
                            out=fc, in0=stB[:, s, 0:F],
                            scalar=w10[:, t:t + 1], in1=fb,
                            op0=Alu.mult, op1=Alu.add)
                        feat = fpool.tile([128, 128], f16, tag="feat")
                        fe_i = nc.vector.scalar_tensor_tensor(
                            out=feat, in0=stB[:, s, F:2 * F],
                            scalar=w11[:, t:t + 1], in1=fc,
                            op0=Alu.mult, op1=Alu.add)
                        if s == GATHER_GROUP - 1:
                            last_comb_of_group[g] = fe_i
                        # PE-mode transpose [pt,f] -> [f,pt] (engine inst:
                        # multi-wait is fine here)
                        tp = ptt.tile([128, 128], f16, tag="tp")
                        nc.tensor.transpose(tp, feat, ident)
                        ftT = fpool.tile([128, 128], f16, tag="ftT")
                        nc.scalar.activation(ftT, tp, Act.Copy)
                        if s % 4 == 0:
                            ph = ptp.tile([128, 512], f32, tag="ph")
                        nc.tensor.matmul(
                            ph[:, (s % 4) * 128:(s % 4 + 1) * 128],
                            lhsT=w1_sb, rhs=ftT, start=True, stop=True)
                        if s % 4 == 3:
                            h1 = fpool.tile([128, 512], f16, tag="h1")
                            nc.scalar.activation(h1, ph, Act.Relu,
                                                 bias=b1_sb[:, 0:1])
                            for u in range(4):
                                tu = t - 3 + u
                                po = ptp4.tile([128, 4], f32, tag="po")
                                nc.tensor.matmul(
                                    po,
                                    lhsT=h1[:, u * 128:(u + 1) * 128],
                                    rhs=w2_sb, start=True, stop=True)
                                last_osb = nc.vector.scalar_tensor_tensor(
                                    out=osb[:, tu * 4:(tu + 1) * 4],
                                    in0=po, scalar=1.0, in1=b2_sb,
                                    op0=Alu.mult, op1=Alu.add)
                pno = _absorb(nc, last_osb)
                _after(pno, nc.gpsimd.dma_start(
                    outd.ap().rearrange("(p t) c -> p (t c)", p=128), osb))
                if gdump is not None:
                    pg = _absorb(nc, st0, st1)
                    _after(pg, nc.gpsimd.dma_start(
                        gdump[:, :, :], gridT[:, :, :]))
                    pg2 = _absorb(nc, w11_i)
                    _after(pg2, nc.gpsimd.dma_start(cdump[:, :], cellA))
                    _after(pg2, nc.gpsimd.dma_start(frdump[:, :], fr))
                    pg3 = _absorb(nc, last_comb_of_group[0])
                    _after(pg3, nc.gpsimd.dma_start(
                        sdump[:, :, :], stpool_first_stA))
    nc.compile()
    return nc


# trace-time constant (b_levels[0]); set by kernel() before building
B_LEVEL0 = [1.0 / 6.0]
DEBUG_DUMP_GRID = [False]


def kernel(pt, base_features, b_levels, W1, b1, W2, b2):
    from concourse.bass_utils import run_bass_kernel_spmd

    pt = np.ascontiguousarray(np.asarray(pt, np.float32))
    X = np.ascontiguousarray(np.asarray(base_features, np.float32))
    b_levels = np.asarray(b_levels, np.float32)
    B_LEVEL0[0] = float(b_levels[0])
    c1, d2 = _build_consts(b_levels)

    nc = _build_bass()

    base = {
        "x": X,
        "w1d": np.ascontiguousarray(np.asarray(W1, np.float32)),
        "b1d": np.ascontiguousarray(np.asarray(b1, np.float32)),
        "w2d": np.ascontiguousarray(np.asarray(W2, np.float32)),
        "b2d": np.ascontiguousarray(np.asarray(b2, np.float32)),
        "c1d": c1,
        "d2d": d2,
    }
    in_maps = []
    for c in range(N_CORES):
        m = dict(base)
        m["ptx"] = np.ascontiguousarray(pt[c * PTS_CORE:(c + 1) * PTS_CORE])
        in_maps.append(m)

    res = run_bass_kernel_spmd(nc, in_maps, core_ids=list(range(N_CORES)))
    if DEBUG_DUMP_GRID[0]:
        kernel._last_gdump = [r["gdump"] for r in res.results]
    return np.concatenate([r["out"] for r in res.results], axis=0)



# revision 2
# speedup vs baseline: 1.0484x; 1.0484x over previous
# Trainium2 Bass kernel for nn_MipMap — v2: f-sharded grid build.
#
# Each core builds G[:, :, 16-feature slice] for the full 256x256 grid
# (all 6 mip levels pre-combined, conv matrices baked at trace time),
# with a PE-transpose between the axis-0 and axis-1 conv passes (all in
# SBUF, no DRAM staging).  Cores then exchange shards: AllToAll (33-row
# halo chunks) -> local DVE f-interleave into a row-pair-duplicated
# layout dupJ[j,i] = [G[j,i] | G[j+1,i]] -> AllGather the 4MB dup
# stripe -> every core holds the 32MB dup grid.  Point phase: points
# host-sorted by cell, ONE 1KB indirect gather per point (4 corners),
# bilinear combine + 2-layer MLP as before.  Host un-permutes outputs.

import numpy as np

R = 256
F = 128
FS = 16                              # features per core
FH = 8                               # features per build half
N_PTS = 262144
N_CORES = 8
PTS_CORE = N_PTS // N_CORES          # 32768
PT_COLS = PTS_CORE // 128            # 256
LEVELS = [1, 2, 3, 4, 5]
GG = 8                               # point-tiles per gather group

# banded M-tiles: (out_start, out_len); segs: (w, sbuf_tile_idx)
P_TILES = [(0, 96), (96, 96), (192, 64)]
SEGS = [(0, 0), (1, 0), (1, 1), (2, 1)]
# psum->sbuf copy splits per M-tile: (dst_tile, dst_lo, dst_hi, src_lo, src_hi)
MSPLIT = {0: [(0, 0, 96, 0, 96)],
          1: [(0, 96, 128, 0, 32), (1, 0, 32, 32, 64), (1, 32, 64, 64, 96)],
          2: [(1, 64, 128, 0, 64)]}


def _gaussian_kernel(M, std):
    n = np.arange(M, dtype=np.float64) - (M - 1) / 2.0
    w = np.exp(-0.5 * (n / std) ** 2)
    return (w / w.sum()).astype(np.float32)


def _conv_matrix(s):
    kern = _gaussian_kernel(s, s / 2.0).astype(np.float64)
    A = np.zeros((R, R), np.float64)
    for z in range(R):
        for k in range(s):
            m = z - s // 2 + k
            if m < 0:
                m = -m
            elif m > R - 1:
                m = 2 * (R - 1) - m
            A[z, m] += kern[k]
    return A


def _seg_lhsT(A):
    """[4, 128, 96]: seg lhsT[r, m] = A[o0+m, t*128+r]."""
    out = np.zeros((4, 128, 96), np.float64)
    for si, (w, t) in enumerate(SEGS):
        o0, olen = P_TILES[w]
        blk = A[o0:o0 + olen, t * 128:(t + 1) * 128]     # [olen, 128]
        out[si, :, :olen] = blk.T
    return out


def _build_consts2(b_levels):
    mats = {l: _conv_matrix(2 ** l) for l in LEVELS}
    c1 = np.stack([_seg_lhsT(mats[l]) for l in LEVELS])          # [5,4,128,96]
    d2 = np.stack([_seg_lhsT(mats[l] * float(b_levels[li + 1]))
                   for li, l in enumerate(LEVELS)])
    i2 = _seg_lhsT(np.eye(R) * float(b_levels[0]))               # [4,128,96]
    return c1.astype(np.float16), d2.astype(np.float16), i2.astype(np.float16)


def _numpy_model2(pt, X, b_levels, W1, b1, W2, b2):
    """Mirror of the device algorithm using the baked seg matrices."""
    c1, d2, i2 = _build_consts2(np.asarray(b_levels, np.float32))
    X16 = np.asarray(X, np.float16).astype(np.float32)           # [i,j,f]
    G = np.zeros((R, R, F), np.float32)                          # G^T [j,i,f]
    for li in range(5):
        # pass1: Y[i_out, j', f] via segs
        Y = np.zeros((R, R, F), np.float32)
        for si, (w, t) in enumerate(SEGS):
            o0, olen = P_TILES[w]
            lhsT = c1[li, si].astype(np.float32)                 # [128, 96]
            xin = X16[t * 128:(t + 1) * 128]                     # [128, j', f]
            Y[o0:o0 + olen] += np.einsum("km,kjf->mjf", lhsT[:, :olen], xin)
        Y = Y.astype(np.float16).astype(np.float32)
        # pass2: G[j,i,f] += sum_j' d2[j,j'] Y^T[j', i, f]
        YT = Y.transpose(1, 0, 2)                                # [j', i, f]
        for si, (w, t) in enumerate(SEGS):
            o0, olen = P_TILES[w]
            lhsT = d2[li, si].astype(np.float32)
            G[o0:o0 + olen] += np.einsum(
                "km,kif->mif", lhsT[:, :olen], YT[t * 128:(t + 1) * 128])
    XT = X16.transpose(1, 0, 2)
    for si, (w, t) in enumerate(SEGS):
        o0, olen = P_TILES[w]
        lhsT = i2[si].astype(np.float32)
        G[o0:o0 + olen] += np.einsum(
            "km,kif->mif", lhsT[:, :olen], XT[t * 128:(t + 1) * 128])
    G = G.astype(np.float16)
    # dup grid: dupJ[j, i] = [G[j,i] | G[min(j+1,255), i]]
    dup = np.zeros((R, R, 2, F), np.float16)
    dup[:, :, 0] = G
    dup[:255, :, 1] = G[1:]
    dup[255, :, 1] = G[255]
    dupf = dup.reshape(R * R, 2 * F)                             # rows 512B
    af = (np.asarray(pt, np.float32) + 1.0) * 127.5
    fr = af - np.floor(af)
    fl = np.floor(af).astype(np.int32)
    c0, c1i = fl[:, 0], fl[:, 1]
    f0, f1 = fr[:, 0], fr[:, 1]
    cell = c1i * 256 + c0
    strip = np.concatenate([dupf[cell], dupf[cell + 1]], axis=1)  # [N, 4F]
    A0, B0 = strip[:, 0:F], strip[:, F:2 * F]
    A1, B1 = strip[:, 2 * F:3 * F], strip[:, 3 * F:4 * F]
    w00 = ((1 - f1) * (1 - f0))[:, None]
    w10 = (f1 * (1 - f0))[:, None]
    w01 = ((1 - f1) * f0)[:, None]
    w11 = (f1 * f0)[:, None]
    feat = (A0 * w00 + B0 * w10 + A1 * w01 + B1 * w11).astype(np.float16)
    z = feat.astype(np.float32) @ np.asarray(W1, np.float32) + b1
    return np.maximum(z, 0.0) @ np.asarray(W2, np.float32) + b2


def _build_bass2():
    import concourse.bass as bass
    import concourse.mybir as mybir
    import concourse.tile as tile
    from concourse import bacc
    from concourse.masks import make_identity
    dep = tile.add_dep_helper

    f32 = mybir.dt.float32
    f16 = mybir.dt.float16
    i32 = mybir.dt.int32
    Alu = mybir.AluOpType
    Act = mybir.ActivationFunctionType

    nc = bacc.Bacc(num_devices=N_CORES)
    xf = nc.dram_tensor("xf", [R, R, FS], f16, kind="ExternalInput")
    xtf = nc.dram_tensor("xtf", [R, R, FS], f16, kind="ExternalInput")
    c1d = nc.dram_tensor("c1d", [5, 4, 128, 96], f16, kind="ExternalInput")
    d2d = nc.dram_tensor("d2d", [5, 4, 128, 96], f16, kind="ExternalInput")
    i2d = nc.dram_tensor("i2d", [4, 128, 96], f16, kind="ExternalInput")
    ptx = nc.dram_tensor("ptx", [PTS_CORE, 2], f32, kind="ExternalInput")
    w1d = nc.dram_tensor("w1d", [F, 128], f32, kind="ExternalInput")
    b1d = nc.dram_tensor("b1d", [128], f32, kind="ExternalInput")
    w2d = nc.dram_tensor("w2d", [128, 4], f32, kind="ExternalInput")
    b2d = nc.dram_tensor("b2d", [4], f32, kind="ExternalInput")
    outd = nc.dram_tensor("out", [PTS_CORE, 4], f32, kind="ExternalOutput")

    groups = [list(range(N_CORES))]

    with tile.TileContext(nc) as tc:
        with (
            tc.tile_pool(name="dram", bufs=1, space="DRAM") as dpool,
            tc.tile_pool(name="consts", bufs=1) as cpool,
            tc.tile_pool(name="gstage", bufs=1) as gpool,
        ):
            # ---- exchange DRAM buffers ----
            a2ain = dpool.tile([8, 33, R * FS], f16, name="a2ain", tag="a2ain")
            a2aout = dpool.tile([8, 33, R * FS], f16, name="a2aout",
                                tag="a2aout")
            agin = dpool.tile([32, R * 2 * F], f16, name="agin", tag="agin")
            dupg = nc.dram_tensor("dupg", [8, 32, R * 2 * F], f16,
                                  addr_space="Shared")

            # ---- constants ----
            c1_sb = [[cpool.tile([128, 96], f16, name=f"c1_{li}_{s}", tag=f"c1_{li}_{s}")
                      for s in range(4)] for li in range(5)]
            d2_sb = [[cpool.tile([128, 96], f16, name=f"d2_{li}_{s}", tag=f"d2_{li}_{s}")
                      for s in range(4)] for li in range(5)]
            i2_sb = [cpool.tile([128, 96], f16, name=f"i2_{s}", tag=f"i2_{s}")
                     for s in range(4)]
            for li in range(5):
                for s in range(4):
                    nc.gpsimd.dma_start(c1_sb[li][s], c1d[li, s])
                    nc.gpsimd.dma_start(d2_sb[li][s], d2d[li, s])
            for s in range(4):
                nc.gpsimd.dma_start(i2_sb[s], i2d[s])
            w1_sb = cpool.tile([128, 128], f16, tag="w1_sb")
            nc.gpsimd.dma_start(w1_sb, w1d[:, :])
            w2_sb = cpool.tile([128, 4], f16, tag="w2_sb")
            nc.gpsimd.dma_start(w2_sb, w2d[:, :])
            b1_sb = cpool.tile([128, 1], f32, tag="b1_sb")
            nc.gpsimd.dma_start(b1_sb, b1d.ap().rearrange("(h o) -> h o", o=1))
            b2_sb = cpool.tile([128, 4], f32, tag="b2_sb")
            b2_bcast = bass.AP(tensor=b2d.ap().tensor, offset=0,
                               ap=[[0, 128], [1, 4]])
            nc.gpsimd.dma_start(b2_sb, b2_bcast)
            ident = cpool.tile([128, 128], f16, tag="ident")
            make_identity(nc, ident)

            xf_sb = [cpool.tile([128, R * FS], f16, name=f"xf{t}", tag=f"xf{t}")
                     for t in range(2)]
            xtf_sb = [cpool.tile([128, R * FS], f16, name=f"xtf{t}", tag=f"xtf{t}")
                      for t in range(2)]
            for t in range(2):
                nc.gpsimd.dma_start(
                    xf_sb[t],
                    xf[t * 128:(t + 1) * 128].rearrange("i j f -> i (j f)"))
                nc.gpsimd.dma_start(
                    xtf_sb[t],
                    xtf[t * 128:(t + 1) * 128].rearrange("j i f -> j (i f)"))

            # G stage [2 jt][128, (i, f16)]
            gsb = [gpool.tile([128, R * FS], f16, name=f"gsb{jt}", tag=f"gsb{jt}")
                   for jt in range(2)]

            # ================= grid build: two f-halves =================
            for h in range(2):
                with (
                    tc.tile_pool(name=f"y{h}", bufs=1) as ypool,
                    tc.tile_pool(name=f"p1p{h}", bufs=4, space="PSUM") as p1p,
                ):
                    # Y[li][t]: [128, (j',f8)]; YT[li][t]: [128, (f8,i)]
                    Y = [[ypool.tile([128, R * FH], f16, name=f"y{li}_{t}", tag=f"y{li}_{t}")
                          for t in range(2)] for li in range(5)]
                    YT = [[ypool.tile([128, R * FH], f16, name=f"yt{li}_{t}", tag=f"yt{li}_{t}")
                           for t in range(2)] for li in range(5)]
                    # ---- pass1 ----
                    for li in range(5):
                        for w in range(3):
                            o0, olen = P_TILES[w]
                            segs = [s for s, (sw, _) in enumerate(SEGS)
                                    if sw == w]
                            for c4 in range(4):
                                ps = p1p.tile([128, 512], f32, tag="ps")
                                for k, s in enumerate(segs):
                                    t = SEGS[s][1]
                                    rhs = xf_sb[t].rearrange(
                                        "p (j f) -> p j f", f=FS)[
                                        :, c4 * 64:(c4 + 1) * 64,
                                        h * FH:(h + 1) * FH]
                                    nc.tensor.matmul(
                                        ps[:olen], lhsT=c1_sb[li][s][:, :olen],
                                        rhs=rhs, start=(k == 0),
                                        stop=(k == len(segs) - 1))
                                for (dt, d0, d1, s0, s1) in MSPLIT[w]:
                                    dst = Y[li][dt].rearrange(
                                        "p (j f) -> p j f", f=FH)[
                                        d0:d1, c4 * 64:(c4 + 1) * 64, :]
                                    if (li + w + c4) % 2 == 0:
                                        nc.vector.tensor_copy(dst,
                                                              ps[s0:s1])
                                    else:
                                        nc.scalar.activation(dst, ps[s0:s1],
                                                             Act.Copy)
                    # ---- transposes: Y[i,(j',f)] -> YT[j',(f,i)] ----
                    with tc.tile_pool(name=f"trp{h}", bufs=4,
                                      space="PSUM") as trp:
                        for li in range(5):
                            for fs in range(FH):
                                for it in range(2):
                                    for jt in range(2):
                                        src = Y[li][it].rearrange(
                                            "p (j f) -> p j f", f=FH)[
                                            :, jt * 128:(jt + 1) * 128, fs]
                                        tp = trp.tile([128, 128], f16,
                                                      tag="tp")
                                        nc.tensor.transpose(tp, src, ident)
                                        dst = YT[li][jt][
                                            :, fs * R + it * 128:
                                            fs * R + (it + 1) * 128]
                                        if (li + fs) % 2 == 0:
                                            nc.vector.tensor_copy(dst, tp)
                                        else:
                                            nc.scalar.activation(dst, tp,
                                                                 Act.Copy)
                    # ---- pass2 ----
                    with tc.tile_pool(name=f"p2p{h}", bufs=4,
                                      space="PSUM") as p2p:
                        for w in range(3):
                            o0, olen = P_TILES[w]
                            segs = [s for s, (sw, _) in enumerate(SEGS)
                                    if sw == w]
                            for c4 in range(4):
                                ps = p2p.tile([128, 512], f32, tag="ps2")
                                n_mm = len(segs) * 6
                                k = 0
                                for li in range(5):
                                    for s in segs:
                                        t = SEGS[s][1]
                                        rhs = YT[li][t].rearrange(
                                            "p (f i) -> p f i", i=R)[
                                            :, :, c4 * 64:(c4 + 1) * 64]
                                        nc.tensor.matmul(
                                            ps[:olen],
                                            lhsT=d2_sb[li][s][:, :olen],
                                            rhs=rhs, start=(k == 0),
                                            stop=False)
                                        k += 1
                                for s in segs:
                                    t = SEGS[s][1]
                                    rhs = xtf_sb[t].rearrange(
                                        "p (i f) -> p f i", f=FS)[
                                        :, h * FH:(h + 1) * FH,
                                        c4 * 64:(c4 + 1) * 64]
                                    k += 1
                                    nc.tensor.matmul(
                                        ps[:olen], lhsT=i2_sb[s][:, :olen],
                                        rhs=rhs, start=False,
                                        stop=(k == n_mm))
                                for (dt, d0, d1, s0, s1) in MSPLIT[w]:
                                    dst = gsb[dt].rearrange(
                                        "p (i f) -> p f i", f=FS)[
                                        d0:d1, h * FH:(h + 1) * FH,
                                        c4 * 64:(c4 + 1) * 64]
                                    if (w + c4) % 2 == 0:
                                        nc.vector.tensor_copy(dst, ps[s0:s1])
                                    else:
                                        nc.scalar.activation(dst, ps[s0:s1],
                                                             Act.Copy)

            # ================= exchange =================
            # a2ain chunk d = G^T rows [32d, 32d+33) of this core's f-slice
            for d in range(8):
                lo = 32 * d
                rows = [(lo, min(lo + 33, 256))]
                if d == 7:
                    rows = [(224, 256), (255, 256)]   # repeat last row
                off = 0
                for (r0, r1) in rows:
                    t0 = r0 // 128
                    t1 = (r1 - 1) // 128
                    if t0 == t1:
                        spans = [(t0, r0 - t0 * 128, r1 - t0 * 128)]
                    else:
                        spans = [(t0, r0 - t0 * 128, 128),
                                 (t1, 0, r1 - 128)]
                    for (t, a, b) in spans:
                        nc.sync.dma_start(
                            a2ain[d, off:off + (b - a), :],
                            gsb[t][a:b, :])
                        off += b - a
            nc.gpsimd.collective_compute(
                "AllToAll", mybir.AluOpType.bypass, replica_groups=groups,
                ins=[a2ain.opt()], outs=[a2aout.opt()])

            with tc.tile_pool(name="ex", bufs=1) as expool:
                for ihalf in range(2):
                    # a2aout[k] rows [e, e+32), i-half slice of the free dim
                    a2h = [[expool.tile([32, 128 * FS], f16,
                                        name=f"a2h{k}_{e}",
                                        tag=f"a2h{k}_{e}")
                            for e in range(2)] for k in range(8)]
                    for k in range(8):
                        for e in range(2):
                            src = a2aout[k].rearrange(
                                "p (i f) -> p i f", f=FS)[
                                e:e + 32,
                                ihalf * 128:(ihalf + 1) * 128, :]
                            nc.sync.dma_start(
                                a2h[k][e].rearrange("p (i f) -> p i f",
                                                    f=FS), src)
                    dsb = expool.tile([32, 32768], f16, name="dsb",
                                      tag="dsb")
                    for k in range(8):
                        for e in range(2):
                            src = a2h[k][e].rearrange(
                                "p (i f) -> p i f", f=FS)
                            dst = dsb.rearrange(
                                "p (i e f) -> p i e f", e=2, f=F)[
                                :, :, e, k * FS:(k + 1) * FS]
                            if (k + e) % 2 == 0:
                                nc.vector.tensor_copy(dst, src)
                            else:
                                nc.scalar.activation(dst, src, Act.Copy)
                    nc.sync.dma_start(
                        agin[:, ihalf * 32768:(ihalf + 1) * 32768], dsb)
            cc2 = nc.gpsimd.collective_compute(
                "AllGather", mybir.AluOpType.bypass, replica_groups=groups,
                ins=[agin.opt()], outs=[dupg.ap().opt()])

            # flat view of dup grid: [65536 rows, 256 f16] (512B rows)
            dupflat = dupg.ap().rearrange("c j (r e) -> (c j r) e",
                                          e=2 * F)

            # ================= point phase =================
            with (
                tc.tile_pool(name="ptw", bufs=1) as wpool,
                tc.tile_pool(name="strips", bufs=2) as stpool,
                tc.tile_pool(name="feat", bufs=3) as fpool,
                tc.tile_pool(name="ptp", bufs=2, space="PSUM") as ptp,
                tc.tile_pool(name="ptt", bufs=2, space="PSUM") as ptt,
                tc.tile_pool(name="ptp4", bufs=2, space="PSUM") as ptp4,
            ):
                pt_sb = wpool.tile([128, PT_COLS * 2], f32, tag="pt_sb")
                nc.sync.dma_start(
                    pt_sb, ptx.ap().rearrange("(p t) c -> p (t c)", p=128))
                af = wpool.tile([128, PT_COLS * 2], f32, tag="af")
                nc.vector.tensor_scalar(af, pt_sb, 1.0, 127.5,
                                        Alu.add, Alu.mult)
                il0 = wpool.tile([128, PT_COLS * 2], i32, tag="il0")
                nc.vector.tensor_copy(il0, af)
                ilf = wpool.tile([128, PT_COLS * 2], f32, tag="ilf")
                nc.vector.tensor_copy(ilf, il0)
                dd = wpool.tile([128, PT_COLS * 2], f32, tag="dd")
                nc.vector.tensor_tensor(dd, af, ilf, Alu.subtract)
                neg = wpool.tile([128, PT_COLS * 2], f32, tag="neg")
                nc.vector.tensor_scalar(neg, dd, 0.0, None, Alu.is_lt)
                fr = wpool.tile([128, PT_COLS * 2], f32, tag="fr")
                nc.vector.tensor_tensor(fr, dd, neg, Alu.add)
                flf = wpool.tile([128, PT_COLS * 2], f32, tag="flf")
                nc.vector.tensor_tensor(flf, ilf, neg, Alu.subtract)
                il = wpool.tile([128, PT_COLS * 2], i32, tag="il")
                nc.vector.tensor_copy(il, flf)
                il3 = il.rearrange("p (t c) -> p t c", c=2)
                fr3 = fr.rearrange("p (t c) -> p t c", c=2)
                cellA = wpool.tile([128, PT_COLS], i32, tag="cellA")
                nc.vector.tensor_scalar(cellA, il3[:, :, 1], 256, None,
                                        Alu.mult)
                nc.vector.tensor_tensor(cellA, cellA, il3[:, :, 0], Alu.add)
                g0 = wpool.tile([128, PT_COLS], f32, tag="g0")
                nc.vector.tensor_scalar(g0, fr3[:, :, 0], -1.0, 1.0,
                                        Alu.mult, Alu.add)
                g1 = wpool.tile([128, PT_COLS], f32, tag="g1")
                nc.vector.tensor_scalar(g1, fr3[:, :, 1], -1.0, 1.0,
                                        Alu.mult, Alu.add)
                w00 = wpool.tile([128, PT_COLS], f32, tag="w00")
                nc.vector.tensor_tensor(w00, g1, g0, Alu.mult)
                w01 = wpool.tile([128, PT_COLS], f32, tag="w01")
                nc.vector.tensor_tensor(w01, g1, fr3[:, :, 0], Alu.mult)
                w10 = wpool.tile([128, PT_COLS], f32, tag="w10")
                nc.vector.tensor_tensor(w10, fr3[:, :, 1], g0, Alu.mult)
                w11 = wpool.tile([128, PT_COLS], f32, tag="w11")
                nc.vector.tensor_tensor(w11, fr3[:, :, 1], fr3[:, :, 0],
                                        Alu.mult)
                osb = wpool.tile([128, PT_COLS * 4], f32, tag="osb")

                n_groups = PT_COLS // GG
                ph = None
                for g in range(n_groups):
                    t0g = g * GG
                    st = stpool.tile([128, GG, 4 * F], f16, tag="st")
                    for s_i in range(GG):
                        gi = nc.gpsimd.indirect_dma_start(
                            out=st[:, s_i, :], out_offset=None,
                            in_=dupflat[:, :],
                            in_offset=bass.IndirectOffsetOnAxis(
                                ap=cellA[:, t0g + s_i:t0g + s_i + 1],
                                axis=0))
                        if g == 0 and s_i == 0:
                            dep(gi.ins, cc2.ins,
                                reason="gathers read AllGathered dup grid")
                    for s in range(GG):
                        t = t0g + s
                        fa = fpool.tile([128, 128], f16, tag="fa")
                        nc.scalar.activation(fa, st[:, s, 0:F], Act.Copy,
                                             scale=w00[:, t:t + 1])
                        fb = fpool.tile([128, 128], f16, tag="fb")
                        nc.vector.scalar_tensor_tensor(
                            out=fb, in0=st[:, s, F:2 * F],
                            scalar=w10[:, t:t + 1], in1=fa,
                            op0=Alu.mult, op1=Alu.add)
                        fc = fpool.tile([128, 128], f16, tag="fc")
                        if s % 2 == 0:
                            nc.scalar.activation(fc, st[:, s, 2 * F:3 * F],
                                                 Act.Copy,
                                                 scale=w01[:, t:t + 1])
                        else:
                            nc.vector.tensor_scalar(fc, st[:, s, 2 * F:3 * F],
                                                    w01[:, t:t + 1], None,
                                                    Alu.mult)
                        fd = fpool.tile([128, 128], f16, tag="fd")
                        nc.vector.scalar_tensor_tensor(
                            out=fd, in0=st[:, s, 3 * F:4 * F],
                            scalar=w11[:, t:t + 1], in1=fc,
                            op0=Alu.mult, op1=Alu.add)
                        feat = fpool.tile([128, 128], f16, tag="feat")
                        nc.vector.tensor_tensor(feat, fb, fd, Alu.add)
                        tp = ptt.tile([128, 128], f16, tag="tp")
                        nc.tensor.transpose(tp, feat, ident)
                        ftT = fpool.tile([128, 128], f16, tag="ftT")
                        if s % 2 == 0:
                            nc.vector.tensor_copy(ftT, tp)
                        else:
                            nc.scalar.activation(ftT, tp, Act.Copy)
                        if s % 4 == 0:
                            ph = ptp.tile([128, 512], f32, tag="ph")
                        nc.tensor.matmul(
                            ph[:, (s % 4) * 128:(s % 4 + 1) * 128],
                            lhsT=w1_sb, rhs=ftT, start=True, stop=True)
                        if s % 4 == 3:
                            h1 = fpool.tile([128, 512], f16, tag="h1")
                            nc.scalar.activation(h1, ph, Act.Relu,
                                                 bias=b1_sb[:, 0:1])
                            for u in range(4):
                                tu = t - 3 + u
                                po = ptp4.tile([128, 4], f32, tag="po")
                                nc.tensor.matmul(
                                    po, lhsT=h1[:, u * 128:(u + 1) * 128],
                                    rhs=w2_sb, start=True, stop=True)
                                nc.vector.scalar_tensor_tensor(
                                    out=osb[:, tu * 4:(tu + 1) * 4],
                                    in0=po, scalar=1.0, in1=b2_sb,
                                    op0=Alu.mult, op1=Alu.add)
                nc.sync.dma_start(
                    outd.ap().rearrange("(p t) c -> p (t c)", p=128), osb)
    nc.compile()
    return nc


def kernel(pt, base_features, b_levels, W1, b1, W2, b2):
    from concourse.bass_utils import run_bass_kernel_spmd

    pt = np.ascontiguousarray(np.asarray(pt, np.float32))
    X16 = np.asarray(base_features, np.float16)
    b_levels = np.asarray(b_levels, np.float32)
    c1, d2, i2 = _build_consts2(b_levels)

    af = (pt.astype(np.float64) + 1.0) * 127.5
    fl = np.floor(af).astype(np.int64)
    key = fl[:, 1] * 256 + fl[:, 0]
    sort_idx = np.argsort(key, kind="stable")
    pts = pt[sort_idx]

    XT16 = np.ascontiguousarray(X16.transpose(1, 0, 2))
    nc = _build_bass2()
    base = {
        "c1d": c1, "d2d": d2, "i2d": i2,
        "w1d": np.ascontiguousarray(np.asarray(W1, np.float32)),
        "b1d": np.ascontiguousarray(np.asarray(b1, np.float32)),
        "w2d": np.ascontiguousarray(np.asarray(W2, np.float32)),
        "b2d": np.ascontiguousarray(np.asarray(b2, np.float32)),
    }
    in_maps = []
    for c in range(N_CORES):
        m = dict(base)
        m["xf"] = np.ascontiguousarray(X16[:, :, c * FS:(c + 1) * FS])
        m["xtf"] = np.ascontiguousarray(XT16[:, :, c * FS:(c + 1) * FS])
        m["ptx"] = np.ascontiguousarray(pts[c * PTS_CORE:(c + 1) * PTS_CORE])
        in_maps.append(m)

    res = run_bass_kernel_spmd(nc, in_maps, core_ids=list(range(N_CORES)))
    sorted_out = np.concatenate([r["out"] for r in res.results], axis=0)
    out = np.empty_like(sorted_out)
    out[sort_idx] = sorted_out
    return out
